# revision 10
# baseline (speedup 1.0000x reference)
"""Trainium2 Bass kernel for the DigitalTwinModel (3-layer LSTM digital twin).

Strategy: 8-way model parallelism over the hidden dimension.
  - All activations live feature-major on chip: [features(part), batch(free)],
    batch N=256 so fp32r matmuls run at full (1 cycle/row) speed.
  - Core k owns hidden-feature slice k*128:(k+1)*128 of every LSTM layer's
    h/c state, and the matching 4x128 rows of W_ih / W_hh (gate order
    i,f,g,o). Weights stay resident in SBUF for the whole kernel
    (~20 MB/core) - nothing is re-streamed from HBM inside the loop.
  - After each layer's elementwise, the 128-row h' slice is AllGathered
    ([128,256] per rank -> [1024,256]) so the next matmul can contract over
    the full hidden dim. 3 AllGathers per timestep; the W_hh @ h_l(t)
    matmuls for step t+1 are issued inside the AllGather windows so the
    TensorEngine stays busy while ncfw moves bytes:
      AG0(t) window: batch-major output write of step t-1
      AG1(t) window: W_hh[0] @ h0(t)
      AG2(t) window: W_hh[1] @ h1(t)
      decoder tail:  W_hh[2] @ h2(t)
  - Decoder (Wd1 relu, Wd2, re-encode We relu) is replicated on every core
    (cheaper than more collectives). The model output is additionally
    computed batch-major via matmul(lhsT=rT, rhs=Wd2T) = r @ Wd2.T so it
    can be DMA'd straight into out[:, t, :] with no transpose; bd2 is
    added on the host.
"""

import numpy as np

import concourse.bass as bass
import concourse.mybir as mybir
from concourse import bacc
import concourse.tile as tile
from concourse.bass_utils import run_bass_kernel_spmd

F32 = mybir.dt.float32
F32R = mybir.dt.float32r
AF = mybir.ActivationFunctionType

B, D_IN, H, L, T = 256, 512, 1024, 3, 32
NCORES = 8
P = 128
SH = H // NCORES          # 128 hidden features owned per core
KT_H = H // P             # 8 k-tiles over hidden dim
KT_D = D_IN // P          # 4 k-tiles over model-output dim
MT_G = 4 * SH // P        # 4 m-tiles of gates per core (one per gate)
GROUPS = [list(range(NCORES))]


def _r(ap):
    """Bitcast an fp32 AP to float32r for full-rate PE matmuls."""
    return ap.bitcast(F32R)


def _pe_touch(nc, ap2d):
    """Tiny ldweights that makes the PE observe a tile's producer semaphore.

    Fused fp32r matmuls have a single sync-wait slot in the ISA; when a
    matmul would need two waits (e.g. fresh-DMA rhs + a PSUM WAR), walrus
    codegen fails. A 1x4 ldweights touch reads the tile on the PE stream
    and absorbs the wait; the junk weights are replaced by the next fused
    matmul's internal weight load.
    """
    nc.tensor.ldweights(weights=ap2d[0:1, 0:2].bitcast(mybir.dt.bfloat16))


def build_program(timesteps=T):
    # default 16 KiB/partition dynamic-DMA scratch is dead weight here (no
    # indirect DMA in this kernel) - reclaim most of it for tiles.
    nc = bacc.Bacc(None, num_devices=NCORES, dynamic_dma_scratch_size=2048)

    # ---- kernel I/O (per-core payloads supplied from the host) ----
    wih = [nc.dram_tensor(f"wih{l}", [H, 4 * SH], F32R, kind="ExternalInput") for l in range(L)]
    whh = [nc.dram_tensor(f"whh{l}", [H, 4 * SH], F32R, kind="ExternalInput") for l in range(L)]
    bg = [nc.dram_tensor(f"bg{l}", [P, MT_G], F32, kind="ExternalInput") for l in range(L)]
    wd1 = nc.dram_tensor("wd1", [H, H], F32R, kind="ExternalInput")
    wd2 = nc.dram_tensor("wd2", [H, D_IN], F32R, kind="ExternalInput")
    we = nc.dram_tensor("we", [D_IN, H], F32R, kind="ExternalInput")
    bd1 = nc.dram_tensor("bd1", [P, KT_H], F32, kind="ExternalInput")
    bd2 = nc.dram_tensor("bd2", [P, KT_D], F32, kind="ExternalInput")
    be = nc.dram_tensor("be", [P, KT_H], F32, kind="ExternalInput")
    enc0 = nc.dram_tensor("enc0", [H, B], F32R, kind="ExternalInput")
    out = nc.dram_tensor("out", [B, timesteps, D_IN], F32, kind="ExternalOutput")

    with tile.TileContext(nc) as tc:
        with (
            tc.tile_pool(name="singles", bufs=1) as singles,
            tc.tile_pool(name="acts", bufs=1) as acts,
            tc.tile_pool(name="gtmp", bufs=1) as gtmp,
            tc.tile_pool(name="hloc", bufs=2) as hlocp,
            tc.tile_pool(name="obuf", bufs=1) as obuf,
            tc.tile_pool(name="pg", bufs=1, space="PSUM") as pgp,
            tc.tile_pool(name="pwork", bufs=2, space="PSUM") as pwork,
            tc.tile_pool(name="dram", bufs=2, space="DRAM") as dram,
        ):
            # ---- load resident weights/biases into SBUF ----
            s_wih, s_whh, s_bg = [], [], []
            for l in range(L):
                w = singles.tile([P, KT_H, 4 * SH], F32R, tag=f"swih{l}", name=f"swih{l}")
                nc.sync.dma_start(out=w, in_=wih[l][:].rearrange("(kk p) m -> p kk m", p=P))
                _pe_touch(nc, w[:, 0, :])
                s_wih.append(w)
            for l in range(L):
                w = singles.tile([P, KT_H, 4 * SH], F32R, tag=f"swhh{l}", name=f"swhh{l}")
                nc.sync.dma_start(out=w, in_=whh[l][:].rearrange("(kk p) m -> p kk m", p=P))
                _pe_touch(nc, w[:, 0, :])
                s_whh.append(w)
            for l in range(L):
                t_ = singles.tile([P, MT_G], F32, tag=f"sbg{l}", name=f"sbg{l}")
                nc.sync.dma_start(out=t_, in_=bg[l][:])
                s_bg.append(t_)
            s_wd1 = singles.tile([P, KT_H, H], F32R, tag="swd1", name="swd1")
            nc.sync.dma_start(out=s_wd1, in_=wd1[:].rearrange("(kk p) m -> p kk m", p=P))
            _pe_touch(nc, s_wd1[:, 0, :])
            s_wd2 = singles.tile([P, KT_H, D_IN], F32R, tag="swd2", name="swd2")
            nc.sync.dma_start(out=s_wd2, in_=wd2[:].rearrange("(kk p) m -> p kk m", p=P))
            _pe_touch(nc, s_wd2[:, 0, :])
            s_we = singles.tile([P, KT_D, H], F32R, tag="swe", name="swe")
            nc.sync.dma_start(out=s_we, in_=we[:].rearrange("(kk p) m -> p kk m", p=P))
            _pe_touch(nc, s_we[:, 0, :])
            s_bd1 = singles.tile([P, KT_H], F32, tag="sbd1", name="sbd1")
            nc.sync.dma_start(out=s_bd1, in_=bd1[:])
            s_bd2 = singles.tile([P, KT_D], F32, tag="sbd2", name="sbd2")
            nc.sync.dma_start(out=s_bd2, in_=bd2[:])
            s_be = singles.tile([P, KT_H], F32, tag="sbe", name="sbe")
            nc.sync.dma_start(out=s_be, in_=be[:])

            # persistent cell state (zero-initialised)
            s_c = []
            for l in range(L):
                c = singles.tile([P, B], F32, tag=f"c{l}", name=f"c{l}")
                nc.vector.memset(c, 0.0)
                s_c.append(c)

            def gate_mms(pg_t, w, rhs, first, last):
                """Accumulate w^T @ rhs into the 4 gate m-tiles of pg_t.

                PSUM `start=True` clears has_written for the whole 2 KiB bank
                (zero region), and two gate m-tiles share each bank - so only
                the bank-FIRST gate (m even) opens the group and only the
                bank-LAST gate (m odd) closes it. The bank-wide clear from the
                even gate's start covers the odd gate's region, whose first
                write then lands in overwrite mode per the has_written bits.
                """
                for m in range(MT_G):
                    for kk in range(KT_H):
                        nc.tensor.matmul(
                            pg_t[:, m, :],
                            lhsT=(w[:, kk, m * P:(m + 1) * P]),
                            rhs=(rhs[:, kk, :]),
                            start=(first and kk == 0 and m % 2 == 0),
                            stop=(last and kk == KT_H - 1 and m % 2 == 1),
                        )

            def elementwise(l, pg_t, first_step):
                """gates -> (h'_k slice, updated c)."""
                ti = gtmp.tile([P, B], F32, tag="ti", name="ti")
                tg = gtmp.tile([P, B], F32, tag="tg", name="tg")
                to = gtmp.tile([P, B], F32, tag="to", name="to")
                nc.scalar.activation(ti, pg_t[:, 0, :], AF.Sigmoid, bias=s_bg[l][:, 0:1])
                if first_step:
                    nc.scalar.activation(tg, pg_t[:, 2, :], AF.Tanh, bias=s_bg[l][:, 2:3])
                    nc.scalar.activation(to, pg_t[:, 3, :], AF.Sigmoid, bias=s_bg[l][:, 3:4])
                    # c = 0 -> c_new = i*g
                    nc.vector.tensor_mul(s_c[l], ti, tg)
                else:
                    tf = gtmp.tile([P, B], F32, tag="tf", name="tf")
                    t1 = gtmp.tile([P, B], F32, tag="t1", name="t1")
                    t2 = gtmp.tile([P, B], F32, tag="t2", name="t2")
                    nc.scalar.activation(tf, pg_t[:, 1, :], AF.Sigmoid, bias=s_bg[l][:, 1:2])
                    nc.scalar.activation(tg, pg_t[:, 2, :], AF.Tanh, bias=s_bg[l][:, 2:3])
                    nc.scalar.activation(to, pg_t[:, 3, :], AF.Sigmoid, bias=s_bg[l][:, 3:4])
                    nc.vector.tensor_mul(t2, tf, s_c[l])   # f * c
                    nc.vector.tensor_mul(t1, ti, tg)       # i * g
                    nc.vector.tensor_add(s_c[l], t1, t2)
                # tanh(c) -> reuse tg slot (dead after i*g)
                tanhc = gtmp.tile([P, B], F32, tag="tg", name="tg")
                nc.scalar.activation(tanhc, s_c[l], AF.Tanh)
                hl = hlocp.tile([P, B], F32R, tag="hl", name="hl")
                nc.vector.tensor_mul(hl, to, tanhc)
                return hl, tanhc

            def allgather(hl, l):
                agin = dram.tile([P, B], F32R, tag=f"agin{l}", name=f"agin{l}")
                agout = dram.tile([H, B], F32R, tag=f"agout{l}", name=f"agout{l}")
                nc.sync.dma_start(out=agin, in_=hl)
                nc.gpsimd.collective_compute(
                    "AllGather",
                    mybir.AluOpType.bypass,
                    replica_groups=GROUPS,
                    ins=[agin.opt()],
                    outs=[agout.opt()],
                )
                hT = acts.tile([P, KT_H, B], F32R, tag=f"hT{l}", name=f"hT{l}")
                nc.sync.dma_start(out=hT, in_=agout[:].rearrange("(kk p) b -> p kk b", p=P))
                return hT

            # rolling state across the unrolled time loop
            enc_t = acts.tile([P, KT_H, B], F32R, tag="encT", name="encT")
            nc.sync.dma_start(out=enc_t, in_=enc0[:].rearrange("(kk p) b -> p kk b", p=P))
            hT = [None] * L        # gathered h_l(t) feature-major
            pg_cur = [None] * L    # psum tiles pre-loaded with W_hh @ h_l(t-1)
            rT_prev = None         # r(t-1) for the deferred batch-major output write
            t_prev = None

            def emit_outbt(rT, tstep):
                """out[:, tstep, :] = (Wd2 @ r)^T via lhsT=rT; bias added on host."""
                ob = obuf.tile([P, B // P, D_IN], F32, tag="ob", name="ob")
                for m in range(B // P):
                    po = pwork.tile([P, D_IN], F32, tag="pw", name="pw")
                    for kk in range(KT_H):
                        nc.tensor.matmul(
                            po,
                            lhsT=(rT[:, kk, m * P:(m + 1) * P]),
                            rhs=(s_wd2[:, kk, :]),
                            start=kk == 0,
                            stop=kk == KT_H - 1,
                        )
                    nc.vector.tensor_copy(out=ob[:, m, :], in_=po)
                nc.sync.dma_start(
                    out=out[:, tstep, :].rearrange("(c p) d -> p c d", p=P),
                    in_=ob,
                )

            for t in range(timesteps):
                first = t == 0

                # ---- layer 0 ----
                if first:
                    pg_cur[0] = pgp.tile([P, MT_G, B], F32, tag="pg0", name="pg0")
                gate_mms(pg_cur[0], s_wih[0], enc_t, first=first, last=True)
                h0l, tanhc0 = elementwise(0, pg_cur[0], first)
                hT0_new = allgather(h0l, 0)
                # AG0 window: deferred batch-major output write of step t-1
                if not first:
                    emit_outbt(rT_prev, t_prev)
                hT[0] = hT0_new

                # ---- layer 1 ----
                if first:
                    pg_cur[1] = pgp.tile([P, MT_G, B], F32, tag="pg1", name="pg1")
                gate_mms(pg_cur[1], s_wih[1], hT[0], first=first, last=True)
                h1l, tanhc1 = elementwise(1, pg_cur[1], first)
                hT1_new = allgather(h1l, 1)
                # AG1 window: W_hh[0] @ h0(t) for step t+1.
                if t + 1 < timesteps:
                    _pe_touch(nc, tanhc0)
                    pg_cur[0] = pgp.tile([P, MT_G, B], F32, tag="pg0", name="pg0")
                    gate_mms(pg_cur[0], s_whh[0], hT[0], first=True, last=False)
                hT[1] = hT1_new

                # ---- layer 2 ----
                if first:
                    pg_cur[2] = pgp.tile([P, MT_G, B], F32, tag="pg2", name="pg2")
                gate_mms(pg_cur[2], s_wih[2], hT[1], first=first, last=True)
                h2l, tanhc2 = elementwise(2, pg_cur[2], first)
                hT2_new = allgather(h2l, 2)
                _pe_touch(nc, hT2_new[:, 0, :])
                # AG2 window: W_hh[1] @ h1(t) for step t+1.
                if t + 1 < timesteps:
                    _pe_touch(nc, tanhc1)
                    pg_cur[1] = pgp.tile([P, MT_G, B], F32, tag="pg1", name="pg1")
                    gate_mms(pg_cur[1], s_whh[1], hT[1], first=True, last=False)
                hT[2] = hT2_new

                # ---- decoder (replicated on every core) ----
                rT = acts.tile([P, KT_H, B], F32R, tag="rT", name="rT")
                for m in range(KT_H):
                    pd = pwork.tile([P, B], F32, tag="pw", name="pw")
                    for kk in range(KT_H):
                        nc.tensor.matmul(
                            pd,
                            lhsT=(s_wd1[:, kk, m * P:(m + 1) * P]),
                            rhs=(hT[2][:, kk, :]),
                            start=kk == 0,
                            stop=kk == KT_H - 1,
                        )
                    nc.scalar.activation(rT[:, m, :], pd, AF.Relu, bias=s_bd1[:, m:m + 1])

                if t + 1 < timesteps:
                    # outT = Wd2 @ rT + bd2 (feature-major, feeds re-encode)
                    outT = acts.tile([P, KT_D, B], F32R, tag="outT", name="outT")
                    for m in range(KT_D):
                        pd = pwork.tile([P, B], F32, tag="pw", name="pw")
                        for kk in range(KT_H):
                            nc.tensor.matmul(
                                pd,
                                lhsT=(s_wd2[:, kk, m * P:(m + 1) * P]),
                                rhs=(rT[:, kk, :]),
                                start=kk == 0,
                                stop=kk == KT_H - 1,
                            )
                        nc.scalar.add(outT[:, m, :], pd, add=s_bd2[:, m:m + 1])
                    # enc(t+1) = relu(We @ outT + be)
                    enc_t = acts.tile([P, KT_H, B], F32R, tag="encT", name="encT")
                    for m in range(KT_H):
                        pd = pwork.tile([P, B], F32, tag="pw", name="pw")
                        for kk in range(KT_D):
                            nc.tensor.matmul(
                                pd,
                                lhsT=(s_we[:, kk, m * P:(m + 1) * P]),
                                rhs=(outT[:, kk, :]),
                                start=kk == 0,
                                stop=kk == KT_D - 1,
                            )
                        nc.scalar.activation(enc_t[:, m, :], pd, AF.Relu, bias=s_be[:, m:m + 1])
                    # decoder-tail window: W_hh[2] @ h2(t) for step t+1.
                    _pe_touch(nc, tanhc2)
                    pg_cur[2] = pgp.tile([P, MT_G, B], F32, tag="pg2", name="pg2")
                    gate_mms(pg_cur[2], s_whh[2], hT[2], first=True, last=False)
                    rT_prev, t_prev = rT, t
                else:
                    emit_outbt(rT, t)

    nc.compile()
    return nc


_CACHE = {}


def _get_program(timesteps):
    if timesteps not in _CACHE:
        _CACHE[timesteps] = build_program(timesteps)
    return _CACHE[timesteps]


def _prep_inputs(x, We, be, W_ih, W_hh, b_ih, b_hh, Wd1, bd1, Wd2, bd2):
    """Host-side layout: shard/transpose weights per core, fold biases."""
    f = np.float32
    x, We, be = np.asarray(x, f), np.asarray(We, f), np.asarray(be, f)
    W_ih, W_hh = np.asarray(W_ih, f), np.asarray(W_hh, f)
    b_ih, b_hh = np.asarray(b_ih, f), np.asarray(b_hh, f)
    Wd1, bd1 = np.asarray(Wd1, f), np.asarray(bd1, f)
    Wd2, bd2 = np.asarray(Wd2, f), np.asarray(bd2, f)

    enc0T = np.ascontiguousarray(np.maximum(x @ We.T + be, 0.0).T)  # [H, B]
    wd1T = np.ascontiguousarray(Wd1.T)
    wd2T = np.ascontiguousarray(Wd2.T)
    weT = np.ascontiguousarray(We.T)
    bd1c = np.ascontiguousarray(bd1.reshape(KT_H, P).T)
    bd2c = np.ascontiguousarray(bd2.reshape(KT_D, P).T)
    bec = np.ascontiguousarray(be.reshape(KT_H, P).T)

    in_maps = []
    for k in range(NCORES):
        rows = np.concatenate(
            [np.arange(g * H + k * SH, g * H + (k + 1) * SH) for g in range(4)]
        )
        m = {
            "wd1": wd1T, "wd2": wd2T, "we": weT,
            "bd1": bd1c, "bd2": bd2c, "be": bec, "enc0": enc0T,
        }
        for l in range(L):
            m[f"wih{l}"] = np.ascontiguousarray(W_ih[l][rows, :].T)
            m[f"whh{l}"] = np.ascontiguousarray(W_hh[l][rows, :].T)
            bsum = (b_ih[l] + b_hh[l])[rows]
            m[f"bg{l}"] = np.ascontiguousarray(bsum.reshape(MT_G, SH).T)
        in_maps.append(m)
    return in_maps, bd2


def kernel(x, We, be, W_ih, W_hh, b_ih, b_hh, Wd1, bd1, Wd2, bd2, timesteps, **run_kw):
    tsteps = int(timesteps)
    nc = _get_program(tsteps)
    in_maps, bd2_np = _prep_inputs(x, We, be, W_ih, W_hh, b_ih, b_hh, Wd1, bd1, Wd2, bd2)
    res = run_bass_kernel_spmd(nc, in_maps, core_ids=list(range(NCORES)), **run_kw)
    kernel.last_results = res
    out = np.asarray(res.results[0]["out"], np.float32) + bd2_np[None, None, :]
    return out


# revision 11
# speedup vs baseline: 6.3215x; 6.3215x over previous
"""Trainium2 Bass kernel for the DigitalTwinModel (3-layer LSTM digital twin).

Strategy: 8-way model parallelism over the hidden dimension.
  - All activations live feature-major on chip: [features(part), batch(free)],
    batch N=256 so fp32r matmuls run at full (1 cycle/row) speed.
  - Core k owns hidden-feature slice k*128:(k+1)*128 of every LSTM layer's
    h/c state, and the matching 4x128 rows of W_ih / W_hh (gate order
    i,f,g,o). Weights stay resident in SBUF for the whole kernel
    (~20 MB/core) - nothing is re-streamed from HBM inside the loop.
  - After each layer's elementwise, the 128-row h' slice is AllGathered
    ([128,256] per rank -> [1024,256]) so the next matmul can contract over
    the full hidden dim. 3 AllGathers per timestep; the W_hh @ h_l(t)
    matmuls for step t+1 are issued inside the AllGather windows so the
    TensorEngine stays busy while ncfw moves bytes:
      AG0(t) window: batch-major output write of step t-1
      AG1(t) window: W_hh[0] @ h0(t)
      AG2(t) window: W_hh[1] @ h1(t)
      decoder tail:  W_hh[2] @ h2(t)
  - Decoder (Wd1 relu, Wd2, re-encode We relu) is replicated on every core
    (cheaper than more collectives). The model output is additionally
    computed batch-major via matmul(lhsT=rT, rhs=Wd2T) = r @ Wd2.T so it
    can be DMA'd straight into out[:, t, :] with no transpose; bd2 is
    added on the host.
"""

import numpy as np

import concourse.bass as bass
import concourse.mybir as mybir
from concourse import bacc
import concourse.tile as tile
from concourse.bass_utils import run_bass_kernel_spmd

F32 = mybir.dt.float32
F32R = mybir.dt.float32r
AF = mybir.ActivationFunctionType

B, D_IN, H, L, T = 256, 512, 1024, 3, 32
NCORES = 8
P = 128
SH = H // NCORES          # 128 hidden features owned per core
KT_H = H // P             # 8 k-tiles over hidden dim
KT_D = D_IN // P          # 4 k-tiles over model-output dim
MT_G = 4 * SH // P        # 4 m-tiles of gates per core (one per gate)
GROUPS = [list(range(NCORES))]


def _r(ap):
    """Bitcast an fp32 AP to float32r for full-rate PE matmuls."""
    return ap.bitcast(F32R)


def _pe_touch(nc, ap2d):
    """Tiny ldweights that makes the PE observe a tile's producer semaphore.

    Fused fp32r matmuls have a single sync-wait slot in the ISA; when a
    matmul would need two waits (e.g. fresh-DMA rhs + a PSUM WAR), walrus
    codegen fails. A 1x4 ldweights touch reads the tile on the PE stream
    and absorbs the wait; the junk weights are replaced by the next fused
    matmul's internal weight load.
    """
    nc.tensor.ldweights(weights=ap2d[0:1, 0:2].bitcast(mybir.dt.bfloat16))


def build_program(timesteps=T):
    # default 16 KiB/partition dynamic-DMA scratch is dead weight here (no
    # indirect DMA in this kernel) - reclaim most of it for tiles.
    nc = bacc.Bacc(None, num_devices=NCORES, dynamic_dma_scratch_size=2048)

    # ---- kernel I/O (per-core payloads supplied from the host) ----
    wih = [nc.dram_tensor(f"wih{l}", [H, 4 * SH], F32R, kind="ExternalInput") for l in range(L)]
    whh = [nc.dram_tensor(f"whh{l}", [H, 4 * SH], F32R, kind="ExternalInput") for l in range(L)]
    bg = [nc.dram_tensor(f"bg{l}", [P, MT_G], F32, kind="ExternalInput") for l in range(L)]
    wd1 = nc.dram_tensor("wd1", [H, H], F32R, kind="ExternalInput")
    wd2 = nc.dram_tensor("wd2", [H, D_IN], F32R, kind="ExternalInput")
    we = nc.dram_tensor("we", [D_IN, H], F32R, kind="ExternalInput")
    bd1 = nc.dram_tensor("bd1", [P, KT_H], F32, kind="ExternalInput")
    bd2 = nc.dram_tensor("bd2", [P, KT_D], F32, kind="ExternalInput")
    be = nc.dram_tensor("be", [P, KT_H], F32, kind="ExternalInput")
    enc0 = nc.dram_tensor("enc0", [H, B], F32R, kind="ExternalInput")
    out = nc.dram_tensor("out", [B, timesteps, D_IN], F32, kind="ExternalOutput")

    with tile.TileContext(nc) as tc:
        with (
            tc.tile_pool(name="singles", bufs=1) as singles,
            tc.tile_pool(name="acts", bufs=1) as acts,
            tc.tile_pool(name="gtmp", bufs=1) as gtmp,
            tc.tile_pool(name="hloc", bufs=2) as hlocp,
            tc.tile_pool(name="obuf", bufs=1) as obuf,
            tc.tile_pool(name="pg", bufs=1, space="PSUM") as pgp,
            tc.tile_pool(name="pwork", bufs=2, space="PSUM") as pwork,
            tc.tile_pool(name="dram", bufs=2, space="DRAM") as dram,
        ):
            # ---- load resident weights/biases into SBUF ----
            s_wih, s_whh, s_bg = [], [], []
            for l in range(L):
                w = singles.tile([P, KT_H, 4 * SH], F32R, tag=f"swih{l}", name=f"swih{l}")
                nc.sync.dma_start(out=w, in_=wih[l][:].rearrange("(kk p) m -> p kk m", p=P))
                _pe_touch(nc, w[:, 0, :])
                s_wih.append(w)
            for l in range(L):
                w = singles.tile([P, KT_H, 4 * SH], F32R, tag=f"swhh{l}", name=f"swhh{l}")
                nc.sync.dma_start(out=w, in_=whh[l][:].rearrange("(kk p) m -> p kk m", p=P))
                _pe_touch(nc, w[:, 0, :])
                s_whh.append(w)
            for l in range(L):
                t_ = singles.tile([P, MT_G], F32, tag=f"sbg{l}", name=f"sbg{l}")
                nc.sync.dma_start(out=t_, in_=bg[l][:])
                s_bg.append(t_)
            s_wd1 = singles.tile([P, KT_H, H], F32R, tag="swd1", name="swd1")
            nc.sync.dma_start(out=s_wd1, in_=wd1[:].rearrange("(kk p) m -> p kk m", p=P))
            _pe_touch(nc, s_wd1[:, 0, :])
            s_wd2 = singles.tile([P, KT_H, D_IN], F32R, tag="swd2", name="swd2")
            nc.sync.dma_start(out=s_wd2, in_=wd2[:].rearrange("(kk p) m -> p kk m", p=P))
            _pe_touch(nc, s_wd2[:, 0, :])
            s_we = singles.tile([P, KT_D, H], F32R, tag="swe", name="swe")
            nc.sync.dma_start(out=s_we, in_=we[:].rearrange("(kk p) m -> p kk m", p=P))
            _pe_touch(nc, s_we[:, 0, :])
            s_bd1 = singles.tile([P, KT_H], F32, tag="sbd1", name="sbd1")
            nc.sync.dma_start(out=s_bd1, in_=bd1[:])
            s_bd2 = singles.tile([P, KT_D], F32, tag="sbd2", name="sbd2")
            nc.sync.dma_start(out=s_bd2, in_=bd2[:])
            s_be = singles.tile([P, KT_H], F32, tag="sbe", name="sbe")
            nc.sync.dma_start(out=s_be, in_=be[:])

            # persistent cell state (zero-initialised)
            s_c = []
            for l in range(L):
                c = singles.tile([P, B], F32, tag=f"c{l}", name=f"c{l}")
                nc.vector.memset(c, 0.0)
                s_c.append(c)

            def gate_mms(pg_t, w, rhs, first, last):
                """Accumulate w^T @ rhs into the 4 gate m-tiles of pg_t.

                PSUM `start=True` clears has_written for the whole 2 KiB bank
                (zero region), and two gate m-tiles share each bank - so only
                the bank-FIRST gate (m even) opens the group and only the
                bank-LAST gate (m odd) closes it. The bank-wide clear from the
                even gate's start covers the odd gate's region, whose first
                write then lands in overwrite mode per the has_written bits.
                """
                for kk in range(KT_H):
                    for m in range(MT_G):
                        nc.tensor.matmul(
                            pg_t[:, m, :],
                            lhsT=(w[:, kk, m * P:(m + 1) * P]),
                            rhs=(rhs[:, kk, :]),
                            start=(first and kk == 0 and m % 2 == 0),
                            stop=(last and kk == KT_H - 1 and m % 2 == 1),
                        )

            def elementwise(l, pg_t, first_step):
                """gates -> (h'_k slice, updated c)."""
                ti = gtmp.tile([P, B], F32, tag="ti", name="ti")
                tg = gtmp.tile([P, B], F32, tag="tg", name="tg")
                to = gtmp.tile([P, B], F32, tag="to", name="to")
                nc.scalar.activation(ti, pg_t[:, 0, :], AF.Sigmoid, bias=s_bg[l][:, 0:1])
                if first_step:
                    nc.scalar.activation(tg, pg_t[:, 2, :], AF.Tanh, bias=s_bg[l][:, 2:3])
                    nc.scalar.activation(to, pg_t[:, 3, :], AF.Sigmoid, bias=s_bg[l][:, 3:4])
                    # c = 0 -> c_new = i*g
                    nc.vector.tensor_mul(s_c[l], ti, tg)
                else:
                    tf = gtmp.tile([P, B], F32, tag="tf", name="tf")
                    t1 = gtmp.tile([P, B], F32, tag="t1", name="t1")
                    t2 = gtmp.tile([P, B], F32, tag="t2", name="t2")
                    nc.scalar.activation(tg, pg_t[:, 2, :], AF.Tanh, bias=s_bg[l][:, 2:3])
                    nc.vector.tensor_mul(t1, ti, tg)       # i * g
                    nc.scalar.activation(tf, pg_t[:, 1, :], AF.Sigmoid, bias=s_bg[l][:, 1:2])
                    nc.vector.tensor_mul(t2, tf, s_c[l])   # f * c
                    nc.scalar.activation(to, pg_t[:, 3, :], AF.Sigmoid, bias=s_bg[l][:, 3:4])
                    nc.vector.tensor_add(s_c[l], t1, t2)
                # tanh(c) -> reuse tg slot (dead after i*g)
                tanhc = gtmp.tile([P, B], F32, tag="tg", name="tg")
                nc.scalar.activation(tanhc, s_c[l], AF.Tanh)
                hl = hlocp.tile([P, B], F32R, tag="hl", name="hl")
                nc.vector.tensor_mul(hl, to, tanhc)
                return hl, tanhc

            def allgather(hl, l):
                agin = dram.tile([P, B], F32R, tag=f"agin{l}", name=f"agin{l}")
                agout = dram.tile([H, B], F32R, tag=f"agout{l}", name=f"agout{l}")
                nc.sync.dma_start(out=agin, in_=hl)
                nc.gpsimd.collective_compute(
                    "AllGather",
                    mybir.AluOpType.bypass,
                    replica_groups=GROUPS,
                    ins=[agin.opt()],
                    outs=[agout.opt()],
                )
                hT = acts.tile([P, KT_H, B], F32R, tag=f"hT{l}", name=f"hT{l}")
                half = KT_H // 2
                nc.sync.dma_start(
                    out=hT[:, 0:half, :],
                    in_=agout[0:half * P, :].rearrange("(kk p) b -> p kk b", p=P))
                nc.sync.dma_start(
                    out=hT[:, half:KT_H, :],
                    in_=agout[half * P:H, :].rearrange("(kk p) b -> p kk b", p=P))
                return hT

            # rolling state across the unrolled time loop
            enc_t = acts.tile([P, KT_H, B], F32R, tag="encT", name="encT")
            nc.sync.dma_start(out=enc_t, in_=enc0[:].rearrange("(kk p) b -> p kk b", p=P))
            hT = [None] * L        # gathered h_l(t) feature-major
            pg_cur = [None] * L    # psum tiles pre-loaded with W_hh @ h_l(t-1)
            rT_prev = None         # r(t-1) for the deferred batch-major output write
            t_prev = None

            def emit_outbt(rT, tstep):
                """out[:, tstep, :] = (Wd2 @ r)^T via lhsT=rT; bias added on host."""
                ob = obuf.tile([P, B // P, D_IN], F32, tag="ob", name="ob")
                for m in range(B // P):
                    po = pwork.tile([P, D_IN], F32, tag="pw", name="pw")
                    for kk in range(KT_H):
                        nc.tensor.matmul(
                            po,
                            lhsT=(rT[:, kk, m * P:(m + 1) * P]),
                            rhs=(s_wd2[:, kk, :]),
                            start=kk == 0,
                            stop=kk == KT_H - 1,
                        )
                    nc.vector.tensor_copy(out=ob[:, m, :], in_=po)
                nc.sync.dma_start(
                    out=out[:, tstep, :].rearrange("(c p) d -> p c d", p=P),
                    in_=ob,
                )

            for t in range(timesteps):
                first = t == 0

                # ---- layer 0 ----
                if first:
                    pg_cur[0] = pgp.tile([P, MT_G, B], F32, tag="pg0", name="pg0")
                gate_mms(pg_cur[0], s_wih[0], enc_t, first=first, last=True)
                h0l, tanhc0 = elementwise(0, pg_cur[0], first)
                hT0_new = allgather(h0l, 0)
                # AG0 window: W_hh[2] @ h2(t-1) for THIS step's L2, plus the
                # deferred batch-major output write of step t-1.
                if not first:
                    _pe_touch(nc, tanhc2_prev)
                    pg_cur[2] = pgp.tile([P, MT_G, B], F32, tag="pg2", name="pg2")
                    gate_mms(pg_cur[2], s_whh[2], hT[2], first=True, last=False)
                    emit_outbt(rT_prev, t_prev)
                hT[0] = hT0_new

                # ---- layer 1 ----
                if first:
                    pg_cur[1] = pgp.tile([P, MT_G, B], F32, tag="pg1", name="pg1")
                gate_mms(pg_cur[1], s_wih[1], hT[0], first=first, last=True)
                h1l, tanhc1 = elementwise(1, pg_cur[1], first)
                hT1_new = allgather(h1l, 1)
                # AG1 window: W_hh[0] @ h0(t) for step t+1.
                if t + 1 < timesteps:
                    _pe_touch(nc, tanhc0)
                    pg_cur[0] = pgp.tile([P, MT_G, B], F32, tag="pg0", name="pg0")
                    gate_mms(pg_cur[0], s_whh[0], hT[0], first=True, last=False)
                hT[1] = hT1_new

                # ---- layer 2 ----
                if first:
                    pg_cur[2] = pgp.tile([P, MT_G, B], F32, tag="pg2", name="pg2")
                gate_mms(pg_cur[2], s_wih[2], hT[1], first=first, last=True)
                h2l, tanhc2 = elementwise(2, pg_cur[2], first)
                hT2_new = allgather(h2l, 2)
                _pe_touch(nc, hT2_new[:, 0, :])
                # AG2 window: W_hh[1] @ h1(t) for step t+1.
                if t + 1 < timesteps:
                    _pe_touch(nc, tanhc1)
                    pg_cur[1] = pgp.tile([P, MT_G, B], F32, tag="pg1", name="pg1")
                    gate_mms(pg_cur[1], s_whh[1], hT[1], first=True, last=False)
                hT[2] = hT2_new

                # ---- decoder (replicated on every core) ----
                rT = acts.tile([P, KT_H, B], F32R, tag="rT", name="rT")
                for m in range(KT_H):
                    pd = pwork.tile([P, B], F32, tag="pw", name="pw")
                    for kk in range(KT_H):
                        nc.tensor.matmul(
                            pd,
                            lhsT=(s_wd1[:, kk, m * P:(m + 1) * P]),
                            rhs=(hT[2][:, kk, :]),
                            start=kk == 0,
                            stop=kk == KT_H - 1,
                        )
                    nc.scalar.activation(rT[:, m, :], pd, AF.Relu, bias=s_bd1[:, m:m + 1])

                if t + 1 < timesteps:
                    # outT = Wd2 @ rT + bd2 (feature-major, feeds re-encode)
                    outT = acts.tile([P, KT_D, B], F32R, tag="outT", name="outT")
                    for m in range(KT_D):
                        pd = pwork.tile([P, B], F32, tag="pw", name="pw")
                        for kk in range(KT_H):
                            nc.tensor.matmul(
                                pd,
                                lhsT=(s_wd2[:, kk, m * P:(m + 1) * P]),
                                rhs=(rT[:, kk, :]),
                                start=kk == 0,
                                stop=kk == KT_H - 1,
                            )
                        nc.scalar.add(outT[:, m, :], pd, add=s_bd2[:, m:m + 1])
                    # enc(t+1) = relu(We @ outT + be)
                    enc_t = acts.tile([P, KT_H, B], F32R, tag="encT", name="encT")
                    for m in range(KT_H):
                        pd = pwork.tile([P, B], F32, tag="pw", name="pw")
                        for kk in range(KT_D):
                            nc.tensor.matmul(
                                pd,
                                lhsT=(s_we[:, kk, m * P:(m + 1) * P]),
                                rhs=(outT[:, kk, :]),
                                start=kk == 0,
                                stop=kk == KT_D - 1,
                            )
                        nc.scalar.activation(enc_t[:, m, :], pd, AF.Relu, bias=s_be[:, m:m + 1])
                    rT_prev, t_prev = rT, t
                    tanhc2_prev = tanhc2
                else:
                    emit_outbt(rT, t)

    nc.compile()
    return nc


_CACHE = {}


def _get_program(timesteps):
    if timesteps not in _CACHE:
        _CACHE[timesteps] = build_program(timesteps)
    return _CACHE[timesteps]


def _prep_inputs(x, We, be, W_ih, W_hh, b_ih, b_hh, Wd1, bd1, Wd2, bd2):
    """Host-side layout: shard/transpose weights per core, fold biases."""
    f = np.float32
    x, We, be = np.asarray(x, f), np.asarray(We, f), np.asarray(be, f)
    W_ih, W_hh = np.asarray(W_ih, f), np.asarray(W_hh, f)
    b_ih, b_hh = np.asarray(b_ih, f), np.asarray(b_hh, f)
    Wd1, bd1 = np.asarray(Wd1, f), np.asarray(bd1, f)
    Wd2, bd2 = np.asarray(Wd2, f), np.asarray(bd2, f)

    enc0T = np.ascontiguousarray(np.maximum(x @ We.T + be, 0.0).T)  # [H, B]
    wd1T = np.ascontiguousarray(Wd1.T)
    wd2T = np.ascontiguousarray(Wd2.T)
    weT = np.ascontiguousarray(We.T)
    bd1c = np.ascontiguousarray(bd1.reshape(KT_H, P).T)
    bd2c = np.ascontiguousarray(bd2.reshape(KT_D, P).T)
    bec = np.ascontiguousarray(be.reshape(KT_H, P).T)

    in_maps = []
    for k in range(NCORES):
        rows = np.concatenate(
            [np.arange(g * H + k * SH, g * H + (k + 1) * SH) for g in range(4)]
        )
        m = {
            "wd1": wd1T, "wd2": wd2T, "we": weT,
            "bd1": bd1c, "bd2": bd2c, "be": bec, "enc0": enc0T,
        }
        for l in range(L):
            m[f"wih{l}"] = np.ascontiguousarray(W_ih[l][rows, :].T)
            m[f"whh{l}"] = np.ascontiguousarray(W_hh[l][rows, :].T)
            bsum = (b_ih[l] + b_hh[l])[rows]
            m[f"bg{l}"] = np.ascontiguousarray(bsum.reshape(MT_G, SH).T)
        in_maps.append(m)
    return in_maps, bd2


def kernel(x, We, be, W_ih, W_hh, b_ih, b_hh, Wd1, bd1, Wd2, bd2, timesteps, **run_kw):
    tsteps = int(timesteps)
    nc = _get_program(tsteps)
    in_maps, bd2_np = _prep_inputs(x, We, be, W_ih, W_hh, b_ih, b_hh, Wd1, bd1, Wd2, bd2)
    res = run_bass_kernel_spmd(nc, in_maps, core_ids=list(range(NCORES)), **run_kw)
    kernel.last_results = res
    out = np.asarray(res.results[0]["out"], np.float32) + bd2_np[None, None, :]
    return out
